# revision 18
# baseline (speedup 1.0000x reference)
"""Trainium2 Bass kernel for AttnApply (sliding-window weighted sum).

out[b, t, c] = sum_i padded[b, t+i, c] * weights[b, t, i]   (T=11, D=5 zero pad)

Strategy
--------
Pure data parallel over batch: 8 cores x 4 batches each.

Per core, the windowed sum is a banded matrix multiply on the TensorEngine,
blocked in 128-ALIGNED time chunks so that every DMA moves large contiguous
per-partition runs (8-17 KB), which is what the SDMA engines need to reach
line rate (~350 GB/s); 512 B descriptor loads measured only ~280 GB/s and
1.6 KB stores ~180 GB/s.

Blocking: output block a covers times [128a, 128a+128); its inputs span
in_pad rows [128a, 128a+138) = all of aligned chunk a plus the first 10 rows
of chunk a+1.  Host supplies in_pad in partition-major chunk layout
in_cb[p, a, c] = in_pad[128a + p, c], so a whole batch loads as ONE DMA with
16.9 KB contiguous per partition.  Each (block, channel-half) is then TWO
accumulating matmuls:

    psum[c, m]  = sum_{k=0}^{127} in_cb[k, a, c] * band_a[k, a, m]   (start)
    psum[c, m] += sum_{k=0}^{9}  in_cb[k, a+1, c] * band_b[k, a, m]  (stop)

with band_a[k, a, m] = w[128a+m, k-m] (0 <= k-m < T) and
band_b[k, a, m] = w[128a+m, 128+k-m], both built host-side in partition-major
layout so each batch's band is one DMA (8 KB/partition).

PSUM partitions are channels (two 128-channel halves); psum [128, 128] f32 is
exactly one PSUM bank.  Results are cast to bf16 into a per-batch output tile
o_bt [128, 2*L] (DVE casts ch0, ACT casts ch1) and stored once per batch as
ONE DMA into channel-major outT [C, L] (8 KB contiguous per partition-row);
host un-transposes and upcasts at the end.

Precision: the kernel is HBM-bandwidth bound and the correctness gate is
rel_err < 2e-2, so all operands travel as plain bf16 and the output is stored
bf16 — rel err ~2.8e-3 measured.

Per rep this is 4 DMAs per batch (in, band_a+b, out) = 16 total, ~21.6 MB.
"""

import contextlib

import ml_dtypes
import numpy as np

import concourse.bass as bass  # noqa: F401  (engine handles hang off nc)
import concourse.mybir as mybir
import concourse.tile as tile
from concourse import bacc
from concourse.bass_utils import run_bass_kernel_spmd

B, L, C, T = 32, 4096, 256, 11
D = T // 2
N_CORES = 8
B_LOC = B // N_CORES            # 4 batches per core
MB = 128                        # output rows per block (aligned)
NBLK = L // MB                  # 32 blocks per batch
KB2 = T - 1                     # 10 spill rows into the next chunk
NCH = NBLK + 1                  # 33 input chunks of 128 rows
LPAD = NCH * MB                 # 4224 padded input rows

_CACHE: dict = {}
LAST_RESULT = None  # BassKernelResults of the most recent run (for test.py)


def _build_nc(repeat: int = 1, bench: bool = False, opts: dict | None = None):
    """Build the bass program. `repeat` re-runs the whole body N times and
    `bench=True` uses internal zero-filled DRAM inputs/outputs with only a
    tiny external "tick" output — both used only for benchmarking; the
    grading path uses repeat=1, bench=False. `opts` selects DMA queue
    assignment / isolation probes."""
    o = {"qin": "sp", "qout": "act", "qband": "alt"}
    o.update(opts or {})

    def _eng(name, i=0):
        if name == "alt":
            name = "sp" if i % 2 == 0 else "act"
        return {"sp": nc.sync, "act": nc.scalar, "gp": nc.gpsimd}[name]

    nc = bacc.Bacc(
        "TRN2",
        target_bir_lowering=False,
        debug=False,
        num_devices=N_CORES,
    )
    kind_in = "Internal" if bench else "ExternalInput"
    kind_out = "Internal" if bench else "ExternalOutput"
    sfx = "_int" if bench else ""
    inp = nc.dram_tensor(
        "in_cb" + sfx, [B_LOC, MB, NCH * C], mybir.dt.bfloat16, kind=kind_in
    ).ap()
    band_a = nc.dram_tensor(
        "band_a" + sfx, [B_LOC, MB, NBLK * MB], mybir.dt.bfloat16, kind=kind_in
    ).ap()
    band_b = nc.dram_tensor(
        "band_b" + sfx, [B_LOC, KB2, NBLK * MB], mybir.dt.bfloat16, kind=kind_in
    ).ap()
    outT = nc.dram_tensor(
        "outT" + sfx, [B_LOC, C, L], mybir.dt.bfloat16, kind=kind_out
    ).ap()
    tick = (
        nc.dram_tensor("tick", [1, C], mybir.dt.float32, kind="ExternalOutput").ap()
        if bench
        else None
    )

    with tile.TileContext(nc) as tc:
        with (
            tc.tile_pool(name="inp", bufs=2) as in_pool,
            tc.tile_pool(name="bnd", bufs=2) as bd_pool,
            tc.tile_pool(name="outp", bufs=2) as o_pool,
            tc.tile_pool(name="ps", bufs=8, space="PSUM") as ps_pool,
        ):
            if bench:
                # back every DRAM page with zeros once per run so reads are
                # real HBM traffic (unbacked-page reads measure absurdly
                # fast and would not represent the grading path)
                with tc.tile_pool(name="z", bufs=1) as z_pool:
                    z = z_pool.tile([MB, NCH * C // 2], mybir.dt.float32, tag="z")
                    nc.gpsimd.memset(z[:, :], 0.0)
                    zb = z[:, :].bitcast(mybir.dt.bfloat16)
                    for b in range(B_LOC):
                        nc.sync.dma_start(out=inp[b], in_=zb[:, : NCH * C])
                        nc.sync.dma_start(out=band_a[b], in_=zb[:, : NBLK * MB])
                        nc.sync.dma_start(
                            out=band_b[b], in_=zb[:KB2, : NBLK * MB]
                        )
                        for ch in range(2):
                            nc.sync.dma_start(
                                out=outT[b, ch * 128 : (ch + 1) * 128, :],
                                in_=zb[:, :L],
                            )

            # repeat via a hardware loop around 8 unrolled bodies: the
            # For_i all-engine barrier costs ~36us/iteration, so amortize
            # it 8x while keeping compile time independent of `repeat`
            # (used only for benchmarking)
            UNROLL = 8
            if repeat > 1:
                assert repeat % UNROLL == 0
                rep_cm, n_un = tc.For_i(0, repeat // UNROLL), UNROLL
            else:
                rep_cm, n_un = contextlib.nullcontext(), 1
            with rep_cm:
              for _un in range(n_un):
                for b in range(B_LOC):
                    # ---- whole-batch loads: 1 input DMA + 2 band DMAs ----
                    in_bt = in_pool.tile([MB, NCH * C], mybir.dt.bfloat16, tag="in")
                    ba_t = bd_pool.tile([MB, NBLK * MB], mybir.dt.bfloat16, tag="ba")
                    bb_t = bd_pool.tile([KB2, NBLK * MB], mybir.dt.bfloat16, tag="bb")
                    if not o.get("stonly"):
                        _eng(o["qin"], b).dma_start(out=in_bt[:, :], in_=inp[b])
                        _eng(o["qband"], b).dma_start(out=ba_t[:, :], in_=band_a[b])
                        _eng(o["qband"], b + 1).dma_start(
                            out=bb_t[:, :], in_=band_b[b]
                        )

                    o_bt = o_pool.tile([128, 2 * L], mybir.dt.bfloat16, tag="o")
                    if o.get("stonly"):
                        nc.vector.memset(o_bt[:, 0:16], 0.0)

                    if not (o.get("ldonly") or o.get("stonly")):
                        PSB = 4  # blocks per psum tile: 4*128 f32 = one bank
                        for g in range(NBLK // PSB):
                            for ch in range(2):
                                ps = ps_pool.tile(
                                    [128, PSB * MB], mybir.dt.float32, tag="ps"
                                )
                                for i in range(PSB):
                                    a = g * PSB + i
                                    c0 = a * C + ch * 128
                                    c1 = (a + 1) * C + ch * 128
                                    nc.tensor.matmul(
                                        ps[:, i * MB : (i + 1) * MB],
                                        in_bt[:, c0 : c0 + 128],
                                        ba_t[:, a * MB : (a + 1) * MB],
                                        start=True,
                                        stop=False,
                                    )
                                    nc.tensor.matmul(
                                        ps[:, i * MB : (i + 1) * MB],
                                        in_bt[:KB2, c1 : c1 + 128],
                                        bb_t[:, a * MB : (a + 1) * MB],
                                        start=False,
                                        stop=True,
                                    )
                                dst = o_bt[
                                    :,
                                    ch * L + g * PSB * MB : ch * L + (g + 1) * PSB * MB,
                                ]
                                if ch == 0:
                                    nc.vector.tensor_copy(out=dst, in_=ps[:, :])
                                else:
                                    nc.scalar.copy(out=dst, in_=ps[:, :])

                    # ---- one whole-batch store (8 KB/partition runs) ----
                    if not o.get("ldonly"):
                        _eng(o["qout"], b + 1).dma_start(
                            out=outT[b].rearrange("(ch p) t -> p ch t", ch=2),
                            in_=o_bt[:, :].rearrange("p (ch t) -> p ch t", ch=2),
                        )
            if tick is not None:
                # flush the HWDGE queues once after all reps: same-queue
                # reads complete only after all prior writes on that queue
                fl = o_pool.tile([3, C], mybir.dt.float32, tag="fl")
                nc.sync.dma_start(
                    out=fl[0:1, : C // 2].bitcast(mybir.dt.bfloat16),
                    in_=outT[0, 0:1, 0:C],
                )
                nc.scalar.dma_start(
                    out=fl[1:2, : C // 2].bitcast(mybir.dt.bfloat16),
                    in_=outT[0, 128:129, 0:C],
                )
                nc.gpsimd.dma_start(
                    out=fl[2:3, : C // 2].bitcast(mybir.dt.bfloat16),
                    in_=outT[0, 64:65, 0:C],
                )
                nc.sync.dma_start(out=tick[:, :], in_=fl[0:1, :])
                nc.sync.dma_start(out=tick[:, 0:C], in_=fl[1:2, :])
                nc.sync.dma_start(out=tick[:, 0:C], in_=fl[2:3, :])
    nc.compile()
    return nc


BF16 = ml_dtypes.bfloat16


def _prep_core(x: np.ndarray, w: np.ndarray):
    """x: [B_LOC, L, C] f32, w: [B_LOC, L, T] f32 ->
    (in_cb, band_a, band_b) in bf16, partition-major layouts."""
    in_pad = np.zeros((B_LOC, LPAD, C), BF16)
    in_pad[:, D : D + L, :] = x.astype(BF16)
    # in_cb[b, p, a, c] = in_pad[b, 128a + p, c]
    in_cb = np.ascontiguousarray(
        in_pad.reshape(B_LOC, NCH, MB, C).transpose(0, 2, 1, 3)
    ).reshape(B_LOC, MB, NCH * C)

    # band_a[b, k, a, m] = w[b, 128a+m, k-m]   (0 <= k-m < T)
    # band_b[b, k, a, m] = w[b, 128a+m, 128+k-m]
    ba = np.zeros((B_LOC, NBLK, MB, MB), np.float32)   # [b, a, k, m]
    bb = np.zeros((B_LOC, NBLK, KB2, MB), np.float32)
    mm = np.arange(MB)
    for tau in range(T):
        kk = mm + tau
        lo = kk < MB
        # rows where k = m + tau < 128 -> band_a
        ba[:, :, kk[lo], mm[lo]] = w[:, :, tau].reshape(B_LOC, NBLK, MB)[
            :, :, mm[lo]
        ]
        # rows where k = m + tau >= 128 -> band_b at k2 = m + tau - 128
        hi = ~lo
        bb[:, :, kk[hi] - MB, mm[hi]] = w[:, :, tau].reshape(B_LOC, NBLK, MB)[
            :, :, mm[hi]
        ]
    band_a = np.ascontiguousarray(ba.transpose(0, 2, 1, 3)).reshape(
        B_LOC, MB, NBLK * MB
    ).astype(BF16)
    band_b = np.ascontiguousarray(bb.transpose(0, 2, 1, 3)).reshape(
        B_LOC, KB2, NBLK * MB
    ).astype(BF16)
    return in_cb, band_a, band_b


def kernel(inputs: np.ndarray, weights: np.ndarray) -> np.ndarray:
    global LAST_RESULT
    inputs = np.ascontiguousarray(np.asarray(inputs, dtype=np.float32))
    weights = np.ascontiguousarray(np.asarray(weights, dtype=np.float32))
    assert inputs.shape == (B, L, C) and weights.shape == (B, L, T)

    if "nc" not in _CACHE:
        _CACHE["nc"] = _build_nc()
    nc = _CACHE["nc"]

    in_maps = []
    for c in range(N_CORES):
        sl = slice(c * B_LOC, (c + 1) * B_LOC)
        ic, ba, bb = _prep_core(inputs[sl], weights[sl])
        in_maps.append({"in_cb": ic, "band_a": ba, "band_b": bb})

    res = run_bass_kernel_spmd(nc, in_maps, core_ids=list(range(N_CORES)))
    LAST_RESULT = res
    # outputs come back channel-major bf16 [B_LOC, C, L]; upcast +
    # un-transpose on host
    return np.ascontiguousarray(
        np.concatenate(
            [
                r["outT"].astype(np.float32).transpose(0, 2, 1)
                for r in res.results
            ],
            axis=0,
        )
    )
